# revision 20
# baseline (speedup 1.0000x reference)
"""Trainium2 Bass kernel for EvaAttention (B=4, S=2048, C=1024, H=16, D=64).

Sharding: 8 cores = 4 batches x 2 head-groups (8 heads each). Each core runs
the identical SPMD program on host-sliced inputs.

v3 design (fp16 qk path, bf16 attention probs, interleaved emission):
  - matmul operands 16-bit (fp16 for the q/k/projection path for precision,
    bf16 for exp outputs for range); PSUM accumulation stays fp32,
  - x^T resident in SBUF (fp16); all weights resident,
  - attention is ACT(exp)-bound at ~1.1us per k-tile; projection matmuls are
    emitted interleaved into the attention stream (1 filler MM per k-tile)
    so the tensor engine uses the exp slack instead of serializing phases,
  - v projection is interleaved into the first attention unit (8 MMs per
    k-tile, just ahead of the AV matmul that consumes each chunk),
  - RoPE: bias applied during PSUM eviction (tensor_scalar), rotate-half by
    partition-block SBUF->SBUF DMAs, combine with two fp16 tensor_tensor ops,
  - softmax denominators ride the AV matmul via a ones-column in v_store;
    normalize = psum-direct reciprocal + gpsimd partition broadcast,
  - output projection as a short tail phase (contraction over all 4 pairs).
Host sums the two head-group partials per batch and adds the bias
corrections (proj bias + v_bias folded through the projection).
"""

import os
import sys
from collections import deque

import numpy as np
import ml_dtypes

for _p in ("/opt/trn_rl_repo", "/root/.axon_site/_ro/trn_rl_repo"):
    if os.path.isdir(_p) and _p not in sys.path:
        sys.path.append(_p)

import concourse.bass as bass  # noqa: E402,F401
import concourse.mybir as mybir  # noqa: E402
import concourse.tile as tile  # noqa: E402
from concourse import bacc  # noqa: E402
from concourse.bass_utils import run_bass_kernel_spmd  # noqa: E402

F32 = mybir.dt.float32
BF16 = mybir.dt.bfloat16
F16 = mybir.dt.float16
AF = mybir.ActivationFunctionType
OP = mybir.AluOpType
BF16NP = ml_dtypes.bfloat16

B = 4
C = 1024
D = 64
H = 16
HPC = 8  # heads per core
NCORES = 8
KC = C // 128  # contraction chunks for the projections
VW = D + 1  # v-store block width per head (64 v cols + ones col)
NCH = 512  # matmul free-dim chunk (one PSUM bank of fp32)


def _emit(tc, io, S):
    nc = tc.nc
    KT = S // 128  # k-position tiles
    S2 = S // 2  # attention q-pass width
    NQ = S // NCH

    with (
        tc.tile_pool(name="cst", bufs=1) as cpool,
        tc.tile_pool(name="xtp", bufs=1) as xt_pool,
        tc.tile_pool(name="wp", bufs=1) as w_pool,
        tc.tile_pool(name="vstp", bufs=1) as v_pool,
        tc.tile_pool(name="qkfp", bufs=1) as qkf_pool,
        tc.tile_pool(name="ropep", bufs=1) as rope_pool,
        tc.tile_pool(name="attnp", bufs=1) as attn_pool,
        tc.tile_pool(name="divp", bufs=1) as div_pool,
        tc.tile_pool(name="outp", bufs=1) as out_pool,
        tc.tile_pool(name="ysbp", bufs=1) as ysb_pool,
        tc.tile_pool(name="psA", bufs=1, space="PSUM") as pA_pool,
        tc.tile_pool(name="psQK", bufs=1, space="PSUM") as qkp_pool,
        tc.tile_pool(name="psAV", bufs=1, space="PSUM") as av_pool,
    ):
        # ---- constants + resident tensors -------------------------------
        qkb_sb = cpool.tile([128, 8], F32, tag="qkb", name="qkb")
        nc.sync.dma_start(out=qkb_sb, in_=io["qkb"])
        wqk_sb = [
            w_pool.tile([128, C], F16, tag="wqk", bufs=KC, name=f"wqk{c}")
            for c in range(KC)
        ]
        for c in range(KC):
            nc.sync.dma_start(
                out=wqk_sb[c][:, 0:256], in_=io["wqkT"][c * 128 : (c + 1) * 128, 0:256]
            )
        xt_sb = [
            xt_pool.tile([128, S], F16, tag="xt", bufs=KC, name=f"xt{c}")
            for c in range(KC)
        ]
        for nj in range(NQ):
            n0 = nj * NCH
            for c in range(KC):
                nc.sync.dma_start(
                    out=xt_sb[c][:, n0 : n0 + NCH],
                    in_=io["xT"][c * 128 : (c + 1) * 128, n0 : n0 + NCH],
                )
        for c in range(KC):
            nc.sync.dma_start(
                out=wqk_sb[c][:, 256:C], in_=io["wqkT"][c * 128 : (c + 1) * 128, 256:C]
            )
        cos2_sb = cpool.tile([128, S], F16, tag="cos2", name="cos2")
        nc.sync.dma_start(out=cos2_sb, in_=io["cos2"])
        sin2_sb = cpool.tile([128, S], F16, tag="sin2", name="sin2")
        nc.sync.dma_start(out=sin2_sb, in_=io["sin2"])
        wv_sb = []
        for c in range(KC):
            w = w_pool.tile([128, HPC * D], F16, tag="wv", bufs=KC, name=f"wv{c}")
            nc.sync.dma_start(out=w, in_=io["wvT"][c * 128 : (c + 1) * 128, :])
            wv_sb.append(w)
        projw_sb = []
        for kc in range(4):
            w = w_pool.tile([128, C], F16, tag="pjw", bufs=4, name=f"pjw{kc}")
            nc.sync.dma_start(out=w, in_=io["projT"][kc * 128 : (kc + 1) * 128, :])
            projw_sb.append(w)

        v_store = v_pool.tile([128, KT * HPC * VW], BF16, tag="vst", name="vst")
        # ones in the per-head denominator columns only (strided, tiny)
        nc.vector.memset(
            v_store.rearrange("p (g u) -> p g u", u=VW)[:, :, D : D + 1], 1.0
        )

        qkf_tiles = {}
        out_pair = [
            out_pool.tile([128, S], F16, tag="pair", bufs=4, name=f"pair{i}")
            for i in range(4)
        ]

        # ---- stream-B generators (filler PE work) -----------------------
        def gen_vproj():
            """v projection; 8 yields per chunk, chunk COMPLETE (incl. the
            v_store eviction) after the 8th — emission-order deps require the
            eviction to precede the AV matmul that reads the chunk."""
            for gi in range(KT):
                pv = pA_pool.tile([128, HPC * D], F32, tag="pa", bufs=2, name="pv")
                for c in range(KC):
                    nc.tensor.matmul(
                        pv,
                        lhsT=xt_sb[c][:, gi * 128 : (gi + 1) * 128],
                        rhs=wv_sb[c],
                        start=(c == 0),
                        stop=(c == KC - 1),
                        skip_group_check=True,
                    )
                    if c < KC - 1:
                        yield
                dst = v_store[:, gi * HPC * VW : (gi + 1) * HPC * VW].rearrange(
                    "p (h u) -> p h u", u=VW
                )[:, :, 0:D]
                nc.vector.tensor_copy(dst, pv.rearrange("p (h u) -> p h u", u=D))
                yield

        def gen_qkproj(p):
            """qk projection + rope for pair p, nj-major and t-interleaved so
            the chunks attention consumes first complete first; one yield per
            512-column chunk group. evict_act: run the biased PSUM eviction on
            the (idle-in-prologue) scalar engine instead of DVE."""
            evict_act = p == 0
            qkf_p, raw_p, rot_p = {}, {}, {}
            for t in (2 * p, 2 * p + 1):
                qkf_p[t] = qkf_pool.tile(
                    [128, S], F16, tag="qkf", bufs=4, name=f"qkf{t}"
                )
                qkf_tiles[t] = qkf_p[t]
                raw_p[t] = rope_pool.tile([128, S], F16, tag="raw", bufs=2, name="raw")
                rot_p[t] = rope_pool.tile([128, S], F16, tag="rot", bufs=2, name="rot")
            for nj in range(NQ):
                n0 = nj * NCH
                sl = slice(n0, n0 + NCH)
                for t in (2 * p, 2 * p + 1):
                    qkf, raw, rot = qkf_p[t], raw_p[t], rot_p[t]
                    pA = pA_pool.tile([128, NCH], F32, tag="pa", bufs=2, name="pA")
                    for c in range(KC):
                        nc.tensor.matmul(
                            pA,
                            lhsT=wqk_sb[c][:, t * 128 : (t + 1) * 128],
                            rhs=xt_sb[c][:, n0 : n0 + NCH],
                            start=(c == 0),
                            stop=(c == KC - 1),
                            skip_group_check=True,
                        )
                        if c < KC - 1:
                            yield
                    if evict_act:
                        nc.scalar.activation(
                            raw[:, sl], pA, AF.Identity, bias=qkb_sb[:, t : t + 1]
                        )
                    else:
                        nc.vector.tensor_scalar_add(
                            raw[:, sl], pA, qkb_sb[:, t : t + 1]
                        )
                    for blk in range(2):
                        b0 = blk * 64
                        nc.sync.dma_start(
                            out=rot[b0 : b0 + 32, sl],
                            in_=raw[b0 + 32 : b0 + 64, sl],
                        )
                        nc.sync.dma_start(
                            out=rot[b0 + 32 : b0 + 64, sl],
                            in_=raw[b0 : b0 + 32, sl],
                        )
                    t2 = rope_pool.tile([128, NCH], F16, tag="t2", bufs=3, name="t2")
                    nc.vector.tensor_mul(qkf[:, sl], raw[:, sl], cos2_sb[:, sl])
                    nc.vector.tensor_mul(t2, rot[:, sl], sin2_sb[:, sl])
                    nc.vector.tensor_add(qkf[:, sl], qkf[:, sl], t2)
                    yield

        outproj_done = set()

        def gen_outproj(m, nj):
            outproj_done.add((m, nj))
            n0 = nj * NCH
            yp = pA_pool.tile([128, NCH], F32, tag="pa", bufs=2, name="yp")
            for kc in range(4):
                nc.tensor.matmul(
                    yp,
                    lhsT=projw_sb[kc][:, m * 128 : (m + 1) * 128],
                    rhs=out_pair[kc][:, n0 : n0 + NCH],
                    start=(kc == 0),
                    stop=(kc == 3),
                    skip_group_check=True,
                )
                if kc < 3:
                    yield
            ysb = ysb_pool.tile([128, NCH], F16, tag="ysb", bufs=3, name="ysb")
            nc.vector.tensor_copy(ysb, yp)
            nc.sync.dma_start(
                out=io["yT"][m * 128 : (m + 1) * 128, n0 : n0 + NCH], in_=ysb
            )
            yield

        def drain(g):
            for _ in g:
                pass

        bq = deque([(1, gen_qkproj(1)), (2, gen_qkproj(2)), (3, gen_qkproj(3))])
        done_pairs = {0}

        def pump(n):
            while n > 0 and bq:
                try:
                    next(bq[0][1])
                    n -= 1
                except StopIteration:
                    done_pairs.add(bq[0][0])
                    bq.popleft()

        # ---- prologue: pair-0 projections (serial) ----------------------
        drain(gen_qkproj(0))
        vgen = gen_vproj()
        if False:  # BISECT: v interleaved
            drain(vgen)

        # ---- attention: ACT-paced; stream B interleaved -----------------
        for p in range(4):
            while p not in done_pairs:
                # pair p's qk tiles must be fully emitted before its attention
                pid, g = bq.popleft()
                drain(g)
                done_pairs.add(pid)
            qT = qkf_tiles[2 * p]
            kT = qkf_tiles[2 * p + 1]
            for qp in range(2):
                q0 = qp * S2
                for lh in range(2):
                    r0 = lh * 64
                    head = 2 * p + lh
                    first_unit = p == 0 and lh == 0 and qp == 0
                    avp = av_pool.tile([D + 1, S2], F32, tag="av", bufs=1, name="av")
                    for i in range(KT):
                        if first_unit:
                            # build v chunk gi=i just ahead of its AV use
                            for _ in range(KC):
                                next(vgen, None)
                            pump(1)
                        else:
                            pump(1)
                        qkp = qkp_pool.tile(
                            [128, S2], F32, tag="qkp", bufs=2, name="qkp"
                        )
                        for nj in range(S2 // NCH):
                            n0 = nj * NCH
                            nc.tensor.matmul(
                                qkp[:, n0 : n0 + NCH],
                                lhsT=kT[r0 : r0 + 64, i * 128 : (i + 1) * 128],
                                rhs=qT[r0 : r0 + 64, q0 + n0 : q0 + n0 + NCH],
                                start=True,
                                stop=True,
                            )
                        at = attn_pool.tile(
                            [128, S2], BF16, tag="attn", bufs=4, name="at"
                        )
                        nc.scalar.activation(at, qkp, AF.Exp, scale=0.125)
                        vsl = v_store[
                            :,
                            i * HPC * VW + head * VW : i * HPC * VW + (head + 1) * VW,
                        ]
                        for nj in range(S2 // NCH):
                            n0 = nj * NCH
                            nc.tensor.matmul(
                                avp[:, n0 : n0 + NCH],
                                lhsT=vsl,
                                rhs=at[:, n0 : n0 + NCH],
                                start=(i == 0),
                                stop=(i == KT - 1),
                                skip_group_check=True,
                            )
                    # evacuate avp fast (frees the single-buffer PSUM slot),
                    # then normalize from SBUF off the AV critical path
                    avsb = div_pool.tile(
                        [D + 1, S2], F32, tag="avsb", bufs=2, name="avsb"
                    )
                    nc.vector.tensor_copy(avsb, avp)
                    stmp0 = div_pool.tile([1, S2], F32, tag="stmp0", bufs=2, name="s0")
                    nc.sync.dma_start(out=stmp0, in_=avsb[D : D + 1, :])
                    nc.vector.reciprocal_approx_fast(stmp0, stmp0)
                    rbc = div_pool.tile([64, S2], F32, tag="rbc", bufs=2, name="rbc")
                    nc.gpsimd.partition_broadcast(rbc, stmp0)
                    outh = div_pool.tile(
                        [64, S2], F16, tag="outh", bufs=2, name="outh"
                    )
                    nc.vector.tensor_mul(outh, avsb[0:D, :], rbc)
                    nc.sync.dma_start(
                        out=out_pair[p][r0 : r0 + 64, q0 : q0 + S2], in_=outh
                    )
                    pump(4)  # unit-boundary ACT bubble -> filler PE work
                    if p == 3 and qp == 0 and lh == 1:
                        # first q-half of out_pair complete for all pairs:
                        # queue its output-projection chunks as filler work
                        for m_ in range(8):
                            for nj_ in range(2):
                                bq.append((None, gen_outproj(m_, nj_)))

        while bq:
            drain(bq.popleft()[1])
        drain(vgen)

        # ---- output projection (remaining chunks, pairwise interleaved) ---
        rem = [
            (m, nj)
            for m in range(8)
            for nj in range(NQ)
            if (m, nj) not in outproj_done
        ]
        gens = deque(gen_outproj(m, nj) for m, nj in rem)
        while gens:
            cur = [gens.popleft() for _ in range(min(2, len(gens)))]
            live = True
            while live:
                live = False
                for g in cur:
                    if next(g, "done") != "done":
                        live = True


def build(S=2048):
    nc = bacc.Bacc("TRN2", target_bir_lowering=False, debug=False)
    io = {
        "xT": nc.dram_tensor("xT", [C, S], F16, kind="ExternalInput").ap(),
        "wqkT": nc.dram_tensor("wqkT", [C, 2 * HPC * D], F16, kind="ExternalInput").ap(),
        "wvT": nc.dram_tensor("wvT", [C, HPC * D], F16, kind="ExternalInput").ap(),
        "projT": nc.dram_tensor("projT", [HPC * D, C], F16, kind="ExternalInput").ap(),
        "cos2": nc.dram_tensor("cos2", [128, S], F16, kind="ExternalInput").ap(),
        "sin2": nc.dram_tensor("sin2", [128, S], F16, kind="ExternalInput").ap(),
        "qkb": nc.dram_tensor("qkb", [128, 8], F32, kind="ExternalInput").ap(),
        "yT": nc.dram_tensor("yT", [C, S], F16, kind="ExternalOutput").ap(),
    }
    with tile.TileContext(nc) as tc:
        _emit(tc, io, S)
    nc.compile()
    return nc


def make_core_inputs(core, x, qkv_w, q_bias, proj_w, rope_sin, rope_cos):
    """Build the host-side sharded/transposed input dict for one core."""
    S = x.shape[1]
    b, hg = core // 2, core % 2
    f32 = np.float32

    xT = np.ascontiguousarray(x[b].T).astype(np.float16)

    blocks = []
    for p in range(4):
        h0 = hg * HPC + 2 * p
        blocks.append(qkv_w[h0 * D : (h0 + 2) * D, :])  # q rows, heads h0, h0+1
        blocks.append(qkv_w[C + h0 * D : C + (h0 + 2) * D, :])  # k rows
    wqkT = np.ascontiguousarray(np.concatenate(blocks, axis=0).T).astype(np.float16)

    wvT = np.ascontiguousarray(
        qkv_w[2 * C + hg * HPC * D : 2 * C + (hg + 1) * HPC * D, :].T
    ).astype(np.float16)
    projT = np.ascontiguousarray(
        proj_w[:, hg * HPC * D : (hg + 1) * HPC * D].T
    ).astype(np.float16)

    c1 = np.ones((D, S), dtype=f32)
    c1[:, 1:] = rope_cos.T
    cos2 = np.ascontiguousarray(np.vstack([c1, c1])).astype(np.float16)
    s1 = np.zeros((D, S), dtype=f32)
    s1[:, 1:] = rope_sin.T
    s1[:32, :] *= -1.0
    sin2 = np.ascontiguousarray(np.vstack([s1, s1])).astype(np.float16)

    qkb = np.zeros((128, 8), dtype=f32)
    for p in range(4):
        h0 = hg * HPC + 2 * p
        qkb[:, 2 * p] = q_bias[h0 * D : (h0 + 2) * D]

    return {
        "xT": xT,
        "wqkT": wqkT,
        "wvT": wvT,
        "projT": projT,
        "cos2": cos2,
        "sin2": sin2,
        "qkb": qkb,
    }


_PROGRAM = {}


def _get_program(S):
    if S not in _PROGRAM:
        _PROGRAM[S] = build(S)
    return _PROGRAM[S]


def combine_outputs(yT_list, x, v_bias, proj_w, proj_b):
    """Sum per-core partials and add the host-folded bias corrections."""
    S = x.shape[1]
    corr = (
        v_bias.astype(np.float64) @ proj_w.T.astype(np.float64)
        + proj_b.astype(np.float64)
    ).astype(np.float32)
    y = np.empty((B, S, C), dtype=np.float32)
    for b in range(B):
        y[b] = (
            yT_list[2 * b].T.astype(np.float32)
            + yT_list[2 * b + 1].T.astype(np.float32)
            + corr
        )
    return y


def kernel(x, qkv_w, q_bias, v_bias, proj_w, proj_b, rope_sin, rope_cos):
    x = np.asarray(x, dtype=np.float32)
    qkv_w = np.asarray(qkv_w, dtype=np.float32)
    q_bias = np.asarray(q_bias, dtype=np.float32)
    v_bias = np.asarray(v_bias, dtype=np.float32)
    proj_w = np.asarray(proj_w, dtype=np.float32)
    proj_b = np.asarray(proj_b, dtype=np.float32)
    rope_sin = np.asarray(rope_sin, dtype=np.float32)
    rope_cos = np.asarray(rope_cos, dtype=np.float32)

    S = x.shape[1]
    in_maps = [
        make_core_inputs(c, x, qkv_w, q_bias, proj_w, rope_sin, rope_cos)
        for c in range(NCORES)
    ]
    nc = _get_program(S)
    res = run_bass_kernel_spmd(nc, in_maps, core_ids=list(range(NCORES)))
    yT_list = [r["yT"] for r in res.results]
    return combine_outputs(yT_list, x, v_bias, proj_w=proj_w, proj_b=proj_b)


# revision 22
# speedup vs baseline: 1.0010x; 1.0010x over previous
"""Trainium2 Bass kernel for EvaAttention (B=4, S=2048, C=1024, H=16, D=64).

Sharding: 8 cores = 4 batches x 2 head-groups (8 heads each). Each core runs
the identical SPMD program on host-sliced inputs.

v3 design (fp16 qk path, bf16 attention probs, interleaved emission):
  - matmul operands 16-bit (fp16 for the q/k/projection path for precision,
    bf16 for exp outputs for range); PSUM accumulation stays fp32,
  - x^T resident in SBUF (fp16); all weights resident,
  - attention is ACT(exp)-bound at ~1.1us per k-tile; projection matmuls are
    emitted interleaved into the attention stream (1 filler MM per k-tile)
    so the tensor engine uses the exp slack instead of serializing phases,
  - v projection is interleaved into the first attention unit (8 MMs per
    k-tile, just ahead of the AV matmul that consumes each chunk),
  - RoPE: bias applied during PSUM eviction (tensor_scalar), rotate-half by
    partition-block SBUF->SBUF DMAs, combine with two fp16 tensor_tensor ops,
  - softmax denominators ride the AV matmul via a ones-column in v_store;
    normalize = psum-direct reciprocal + gpsimd partition broadcast,
  - output projection as a short tail phase (contraction over all 4 pairs).
Host sums the two head-group partials per batch and adds the bias
corrections (proj bias + v_bias folded through the projection).
"""

import os
import sys
from collections import deque

import numpy as np
import ml_dtypes

for _p in ("/opt/trn_rl_repo", "/root/.axon_site/_ro/trn_rl_repo"):
    if os.path.isdir(_p) and _p not in sys.path:
        sys.path.append(_p)

import concourse.bass as bass  # noqa: E402,F401
import concourse.mybir as mybir  # noqa: E402
import concourse.tile as tile  # noqa: E402
from concourse import bacc  # noqa: E402
from concourse.bass_utils import run_bass_kernel_spmd  # noqa: E402

F32 = mybir.dt.float32
BF16 = mybir.dt.bfloat16
F16 = mybir.dt.float16
AF = mybir.ActivationFunctionType
OP = mybir.AluOpType
BF16NP = ml_dtypes.bfloat16

B = 4
C = 1024
D = 64
H = 16
HPC = 8  # heads per core
NCORES = 8
KC = C // 128  # contraction chunks for the projections
VW = D + 1  # v-store block width per head (64 v cols + ones col)
NCH = 512  # matmul free-dim chunk (one PSUM bank of fp32)


def _emit(tc, io, S):
    nc = tc.nc
    KT = S // 128  # k-position tiles
    S2 = S // 2  # attention q-pass width
    NQ = S // NCH

    with (
        tc.tile_pool(name="cst", bufs=1) as cpool,
        tc.tile_pool(name="xtp", bufs=1) as xt_pool,
        tc.tile_pool(name="wp", bufs=1) as w_pool,
        tc.tile_pool(name="vstp", bufs=1) as v_pool,
        tc.tile_pool(name="qkfp", bufs=1) as qkf_pool,
        tc.tile_pool(name="ropep", bufs=1) as rope_pool,
        tc.tile_pool(name="attnp", bufs=1) as attn_pool,
        tc.tile_pool(name="divp", bufs=1) as div_pool,
        tc.tile_pool(name="outp", bufs=1) as out_pool,
        tc.tile_pool(name="ysbp", bufs=1) as ysb_pool,
        tc.tile_pool(name="psA", bufs=1, space="PSUM") as pA_pool,
        tc.tile_pool(name="psQK", bufs=1, space="PSUM") as qkp_pool,
        tc.tile_pool(name="psAV", bufs=1, space="PSUM") as av_pool,
    ):
        # ---- constants + resident tensors -------------------------------
        qkb_sb = cpool.tile([128, 8], F32, tag="qkb", name="qkb")
        nc.sync.dma_start(out=qkb_sb, in_=io["qkb"])
        wqk_sb = [
            w_pool.tile([128, C], F16, tag="wqk", bufs=KC, name=f"wqk{c}")
            for c in range(KC)
        ]
        for c in range(KC):
            nc.sync.dma_start(
                out=wqk_sb[c][:, 0:256], in_=io["wqkT"][c * 128 : (c + 1) * 128, 0:256]
            )
        xt_sb = [
            xt_pool.tile([128, S], F16, tag="xt", bufs=KC, name=f"xt{c}")
            for c in range(KC)
        ]
        for nj in range(NQ):
            n0 = nj * NCH
            for c in range(KC):
                nc.sync.dma_start(
                    out=xt_sb[c][:, n0 : n0 + NCH],
                    in_=io["xT"][c * 128 : (c + 1) * 128, n0 : n0 + NCH],
                )
        for c in range(KC):
            nc.sync.dma_start(
                out=wqk_sb[c][:, 256:C], in_=io["wqkT"][c * 128 : (c + 1) * 128, 256:C]
            )
        cos2_sb = cpool.tile([128, S], F16, tag="cos2", name="cos2")
        nc.sync.dma_start(out=cos2_sb, in_=io["cos2"])
        sin2_sb = cpool.tile([128, S], F16, tag="sin2", name="sin2")
        nc.sync.dma_start(out=sin2_sb, in_=io["sin2"])
        wv_sb = []
        for c in range(KC):
            w = w_pool.tile([128, HPC * D], F16, tag="wv", bufs=KC, name=f"wv{c}")
            nc.sync.dma_start(out=w, in_=io["wvT"][c * 128 : (c + 1) * 128, :])
            wv_sb.append(w)
        projw_sb = []
        for kc in range(4):
            w = w_pool.tile([128, C], F16, tag="pjw", bufs=4, name=f"pjw{kc}")
            nc.sync.dma_start(out=w, in_=io["projT"][kc * 128 : (kc + 1) * 128, :])
            projw_sb.append(w)

        v_store = v_pool.tile([128, KT * HPC * VW], BF16, tag="vst", name="vst")
        # ones in the per-head denominator columns only (strided, tiny)
        nc.vector.memset(
            v_store.rearrange("p (g u) -> p g u", u=VW)[:, :, D : D + 1], 1.0
        )

        qkf_tiles = {}
        out_pair = [
            out_pool.tile([128, S], F16, tag="pair", bufs=4, name=f"pair{i}")
            for i in range(4)
        ]

        # ---- stream-B generators (filler PE work) -----------------------
        def gen_vproj():
            """v projection; 8 yields per chunk, chunk COMPLETE (incl. the
            v_store eviction) after the 8th — emission-order deps require the
            eviction to precede the AV matmul that reads the chunk."""
            for gi in range(KT):
                pv = pA_pool.tile([128, HPC * D], F32, tag="pa", bufs=2, name="pv")
                for c in range(KC):
                    nc.tensor.matmul(
                        pv,
                        lhsT=xt_sb[c][:, gi * 128 : (gi + 1) * 128],
                        rhs=wv_sb[c],
                        start=(c == 0),
                        stop=(c == KC - 1),
                        skip_group_check=True,
                    )
                    if c < KC - 1:
                        yield
                dst = v_store[:, gi * HPC * VW : (gi + 1) * HPC * VW].rearrange(
                    "p (h u) -> p h u", u=VW
                )[:, :, 0:D]
                nc.vector.tensor_copy(dst, pv.rearrange("p (h u) -> p h u", u=D))
                yield

        def gen_qkproj(p):
            """qk projection + rope for pair p, nj-major and t-interleaved so
            the chunks attention consumes first complete first; one yield per
            512-column chunk group. evict_act: run the biased PSUM eviction on
            the (idle-in-prologue) scalar engine instead of DVE."""
            evict_act = p == 0
            qkf_p, raw_p, rot_p = {}, {}, {}
            for t in (2 * p, 2 * p + 1):
                qkf_p[t] = qkf_pool.tile(
                    [128, S], F16, tag="qkf", bufs=4, name=f"qkf{t}"
                )
                qkf_tiles[t] = qkf_p[t]
                raw_p[t] = rope_pool.tile([128, S], F16, tag="raw", bufs=2, name="raw")
                rot_p[t] = rope_pool.tile([128, S], F16, tag="rot", bufs=2, name="rot")
            if p == 0:
                # k tile first (attention needs all of k, only half of q early)
                order = [(nj, 2 * p + 1) for nj in range(NQ)] + [
                    (nj, 2 * p) for nj in range(NQ)
                ]
            else:
                order = [(nj, t) for nj in range(NQ) for t in (2 * p, 2 * p + 1)]
            for nj, t in order:
                n0 = nj * NCH
                sl = slice(n0, n0 + NCH)
                if True:
                    qkf, raw, rot = qkf_p[t], raw_p[t], rot_p[t]
                    pA = pA_pool.tile([128, NCH], F32, tag="pa", bufs=2, name="pA")
                    for c in range(KC):
                        nc.tensor.matmul(
                            pA,
                            lhsT=wqk_sb[c][:, t * 128 : (t + 1) * 128],
                            rhs=xt_sb[c][:, n0 : n0 + NCH],
                            start=(c == 0),
                            stop=(c == KC - 1),
                            skip_group_check=True,
                        )
                        if c < KC - 1:
                            yield
                    if evict_act:
                        nc.scalar.activation(
                            raw[:, sl], pA, AF.Identity, bias=qkb_sb[:, t : t + 1]
                        )
                    else:
                        nc.vector.tensor_scalar_add(
                            raw[:, sl], pA, qkb_sb[:, t : t + 1]
                        )
                    for blk in range(2):
                        b0 = blk * 64
                        nc.sync.dma_start(
                            out=rot[b0 : b0 + 32, sl],
                            in_=raw[b0 + 32 : b0 + 64, sl],
                        )
                        nc.sync.dma_start(
                            out=rot[b0 + 32 : b0 + 64, sl],
                            in_=raw[b0 : b0 + 32, sl],
                        )
                    t2 = rope_pool.tile([128, NCH], F16, tag="t2", bufs=3, name="t2")
                    nc.vector.tensor_mul(qkf[:, sl], raw[:, sl], cos2_sb[:, sl])
                    nc.vector.tensor_mul(t2, rot[:, sl], sin2_sb[:, sl])
                    nc.vector.tensor_add(qkf[:, sl], qkf[:, sl], t2)
                    yield

        outproj_done = set()

        def gen_outproj(m, nj):
            outproj_done.add((m, nj))
            n0 = nj * NCH
            yp = pA_pool.tile([128, NCH], F32, tag="pa", bufs=2, name="yp")
            for kc in range(4):
                nc.tensor.matmul(
                    yp,
                    lhsT=projw_sb[kc][:, m * 128 : (m + 1) * 128],
                    rhs=out_pair[kc][:, n0 : n0 + NCH],
                    start=(kc == 0),
                    stop=(kc == 3),
                    skip_group_check=True,
                )
                if kc < 3:
                    yield
            ysb = ysb_pool.tile([128, NCH], F16, tag="ysb", bufs=3, name="ysb")
            nc.vector.tensor_copy(ysb, yp)
            nc.sync.dma_start(
                out=io["yT"][m * 128 : (m + 1) * 128, n0 : n0 + NCH], in_=ysb
            )
            yield

        def drain(g):
            for _ in g:
                pass

        bq = deque([(1, gen_qkproj(1)), (2, gen_qkproj(2)), (3, gen_qkproj(3))])
        done_pairs = set()

        def pump(n):
            while n > 0 and bq:
                try:
                    next(bq[0][1])
                    n -= 1
                except StopIteration:
                    done_pairs.add(bq[0][0])
                    bq.popleft()

        # ---- prologue: pair-0 projections (k tile + first q half) -------
        g0 = gen_qkproj(0)
        for _ in range(6 * KC):
            next(g0, None)
        bq.appendleft((0, g0))
        vgen = gen_vproj()
        if False:  # BISECT: v interleaved
            drain(vgen)

        # ---- attention: ACT-paced; stream B interleaved -----------------
        for p in range(4):
            while p > 0 and p not in done_pairs:
                # pair p's qk tiles must be fully emitted before its attention
                pid, g = bq.popleft()
                drain(g)
                done_pairs.add(pid)
            qT = qkf_tiles[2 * p]
            kT = qkf_tiles[2 * p + 1]
            for qp in range(2):
                if qp == 1:
                    while p not in done_pairs:
                        pid, g = bq.popleft()
                        drain(g)
                        done_pairs.add(pid)
                q0 = qp * S2
                for lh in range(2):
                    r0 = lh * 64
                    head = 2 * p + lh
                    first_unit = p == 0 and lh == 0 and qp == 0
                    avp = av_pool.tile([D + 1, S2], F32, tag="av", bufs=1, name="av")
                    for i in range(KT):
                        if first_unit:
                            # build v chunk gi=i just ahead of its AV use
                            for _ in range(KC):
                                next(vgen, None)
                            pump(1)
                        else:
                            pump(2 if qp == 0 else 1)
                        qkp = qkp_pool.tile(
                            [128, S2], F32, tag="qkp", bufs=2, name="qkp"
                        )
                        for nj in range(S2 // NCH):
                            n0 = nj * NCH
                            nc.tensor.matmul(
                                qkp[:, n0 : n0 + NCH],
                                lhsT=kT[r0 : r0 + 64, i * 128 : (i + 1) * 128],
                                rhs=qT[r0 : r0 + 64, q0 + n0 : q0 + n0 + NCH],
                                start=True,
                                stop=True,
                            )
                        at = attn_pool.tile(
                            [128, S2], BF16, tag="attn", bufs=4, name="at"
                        )
                        nc.scalar.activation(at, qkp, AF.Exp, scale=0.125)
                        vsl = v_store[
                            :,
                            i * HPC * VW + head * VW : i * HPC * VW + (head + 1) * VW,
                        ]
                        for nj in range(S2 // NCH):
                            n0 = nj * NCH
                            nc.tensor.matmul(
                                avp[:, n0 : n0 + NCH],
                                lhsT=vsl,
                                rhs=at[:, n0 : n0 + NCH],
                                start=(i == 0),
                                stop=(i == KT - 1),
                                skip_group_check=True,
                            )
                    # evacuate avp fast (frees the single-buffer PSUM slot),
                    # then normalize from SBUF off the AV critical path
                    avsb = div_pool.tile(
                        [D + 1, S2], F32, tag="avsb", bufs=2, name="avsb"
                    )
                    nc.vector.tensor_copy(avsb, avp)
                    stmp0 = div_pool.tile([1, S2], F32, tag="stmp0", bufs=2, name="s0")
                    nc.sync.dma_start(out=stmp0, in_=avsb[D : D + 1, :])
                    nc.vector.reciprocal_approx_fast(stmp0, stmp0)
                    rbc = div_pool.tile([64, S2], F32, tag="rbc", bufs=2, name="rbc")
                    nc.gpsimd.partition_broadcast(rbc, stmp0)
                    outh = div_pool.tile(
                        [64, S2], F16, tag="outh", bufs=2, name="outh"
                    )
                    nc.vector.tensor_mul(outh, avsb[0:D, :], rbc)
                    nc.sync.dma_start(
                        out=out_pair[p][r0 : r0 + 64, q0 : q0 + S2], in_=outh
                    )
                    pump(3)  # unit-boundary ACT bubble -> filler PE work
                    if p == 3 and qp == 0 and lh == 1:
                        # first q-half of out_pair complete for all pairs:
                        # queue its output-projection chunks as filler work
                        for m_ in range(8):
                            for nj_ in range(2):
                                bq.append((None, gen_outproj(m_, nj_)))

        while bq:
            drain(bq.popleft()[1])
        drain(vgen)

        # ---- output projection (remaining chunks, pairwise interleaved) ---
        rem = [
            (m, nj)
            for m in range(8)
            for nj in range(NQ)
            if (m, nj) not in outproj_done
        ]
        gens = deque(gen_outproj(m, nj) for m, nj in rem)
        while gens:
            cur = [gens.popleft() for _ in range(min(2, len(gens)))]
            live = True
            while live:
                live = False
                for g in cur:
                    if next(g, "done") != "done":
                        live = True


def build(S=2048):
    nc = bacc.Bacc("TRN2", target_bir_lowering=False, debug=False)
    io = {
        "xT": nc.dram_tensor("xT", [C, S], F16, kind="ExternalInput").ap(),
        "wqkT": nc.dram_tensor("wqkT", [C, 2 * HPC * D], F16, kind="ExternalInput").ap(),
        "wvT": nc.dram_tensor("wvT", [C, HPC * D], F16, kind="ExternalInput").ap(),
        "projT": nc.dram_tensor("projT", [HPC * D, C], F16, kind="ExternalInput").ap(),
        "cos2": nc.dram_tensor("cos2", [128, S], F16, kind="ExternalInput").ap(),
        "sin2": nc.dram_tensor("sin2", [128, S], F16, kind="ExternalInput").ap(),
        "qkb": nc.dram_tensor("qkb", [128, 8], F32, kind="ExternalInput").ap(),
        "yT": nc.dram_tensor("yT", [C, S], F16, kind="ExternalOutput").ap(),
    }
    with tile.TileContext(nc) as tc:
        _emit(tc, io, S)
    nc.compile()
    return nc


def make_core_inputs(core, x, qkv_w, q_bias, proj_w, rope_sin, rope_cos):
    """Build the host-side sharded/transposed input dict for one core."""
    S = x.shape[1]
    b, hg = core // 2, core % 2
    f32 = np.float32

    xT = np.ascontiguousarray(x[b].T).astype(np.float16)

    blocks = []
    for p in range(4):
        h0 = hg * HPC + 2 * p
        blocks.append(qkv_w[h0 * D : (h0 + 2) * D, :])  # q rows, heads h0, h0+1
        blocks.append(qkv_w[C + h0 * D : C + (h0 + 2) * D, :])  # k rows
    wqkT = np.ascontiguousarray(np.concatenate(blocks, axis=0).T).astype(np.float16)

    wvT = np.ascontiguousarray(
        qkv_w[2 * C + hg * HPC * D : 2 * C + (hg + 1) * HPC * D, :].T
    ).astype(np.float16)
    projT = np.ascontiguousarray(
        proj_w[:, hg * HPC * D : (hg + 1) * HPC * D].T
    ).astype(np.float16)

    c1 = np.ones((D, S), dtype=f32)
    c1[:, 1:] = rope_cos.T
    cos2 = np.ascontiguousarray(np.vstack([c1, c1])).astype(np.float16)
    s1 = np.zeros((D, S), dtype=f32)
    s1[:, 1:] = rope_sin.T
    s1[:32, :] *= -1.0
    sin2 = np.ascontiguousarray(np.vstack([s1, s1])).astype(np.float16)

    qkb = np.zeros((128, 8), dtype=f32)
    for p in range(4):
        h0 = hg * HPC + 2 * p
        qkb[:, 2 * p] = q_bias[h0 * D : (h0 + 2) * D]

    return {
        "xT": xT,
        "wqkT": wqkT,
        "wvT": wvT,
        "projT": projT,
        "cos2": cos2,
        "sin2": sin2,
        "qkb": qkb,
    }


_PROGRAM = {}


def _get_program(S):
    if S not in _PROGRAM:
        _PROGRAM[S] = build(S)
    return _PROGRAM[S]


def combine_outputs(yT_list, x, v_bias, proj_w, proj_b):
    """Sum per-core partials and add the host-folded bias corrections."""
    S = x.shape[1]
    corr = (
        v_bias.astype(np.float64) @ proj_w.T.astype(np.float64)
        + proj_b.astype(np.float64)
    ).astype(np.float32)
    y = np.empty((B, S, C), dtype=np.float32)
    for b in range(B):
        y[b] = (
            yT_list[2 * b].T.astype(np.float32)
            + yT_list[2 * b + 1].T.astype(np.float32)
            + corr
        )
    return y


def kernel(x, qkv_w, q_bias, v_bias, proj_w, proj_b, rope_sin, rope_cos):
    x = np.asarray(x, dtype=np.float32)
    qkv_w = np.asarray(qkv_w, dtype=np.float32)
    q_bias = np.asarray(q_bias, dtype=np.float32)
    v_bias = np.asarray(v_bias, dtype=np.float32)
    proj_w = np.asarray(proj_w, dtype=np.float32)
    proj_b = np.asarray(proj_b, dtype=np.float32)
    rope_sin = np.asarray(rope_sin, dtype=np.float32)
    rope_cos = np.asarray(rope_cos, dtype=np.float32)

    S = x.shape[1]
    in_maps = [
        make_core_inputs(c, x, qkv_w, q_bias, proj_w, rope_sin, rope_cos)
        for c in range(NCORES)
    ]
    nc = _get_program(S)
    res = run_bass_kernel_spmd(nc, in_maps, core_ids=list(range(NCORES)))
    yT_list = [r["yT"] for r in res.results]
    return combine_outputs(yT_list, x, v_bias, proj_w=proj_w, proj_b=proj_b)
